# revision 38
# baseline (speedup 1.0000x reference)
"""Trainium2 Bass kernel for GQA sliding-window attention (8-core SPMD).

Problem: B=8, S=32, D=4096, H=32 Q-heads, KVH=8 KV-heads, HD=128,
sliding window 4096 with 4064 cached positions.

Sharding: tensor-parallel over heads. Core c owns Q heads 4c..4c+3 and KV
head c (one GQA group): Wq/Wk/Wv column-sharded, cache sharded by KV head,
x replicated. Each core computes its 4 heads' attention output in two
head-pair passes; after each pass the cores AllGather that pass's (bf16)
attention outputs so the gather overlaps the next pass's compute. Each core
then applies a column slice of Wo, and the host concatenates column slices
(no host-side arithmetic).

Compute is bf16 on the TensorEngine (fp32 PSUM accumulation, fp32 softmax
intermediates). Host-side sharding also does layout prep:
  - x is fed transposed (xT [D, 256]) so QKV projections produce Q^T/K^T
    directly in [head_dim, token] layout.
  - All large inputs are pre-permuted on the host so every DMA writes each
    SBUF partition from one contiguous DRAM line (2-16KB), not 512B strides.
  - Wq/Wk columns (and cached K's hd axis) are permuted so RoPE's interleaved
    (even,odd) pairs become contiguous halves [0:64]=real, [64:128]=imag.
    The permutation cancels in q.k since both sides share it; V/Wo stay
    unpermuted.
  - SCALE = HD^-0.5 is folded into Wq.

The new-token scores are folded into the cached-KV chunk loop as two extra
128-token chunks with a host-built block-diagonal mask (-1e30 off-diagonal,
user mask on the diagonal), so the whole softmax runs as one uniform
34-chunk software pipeline (lag-2: exp of chunk t overlaps scores of t+1
and sum/AV of t-2).

Softmax skips max-subtraction (scores are O(10), exp is safe in fp32) and
normalization is deferred: unnormalized exp(scores) feeds attn@V, row sums
come from a ones-vector matmul, and 1/sum is applied when copying the
attention output out of PSUM.
"""

import os
import sys
from contextlib import ExitStack

import numpy as np
import ml_dtypes

import concourse.bass as bass
import concourse.tile as tile
import concourse.mybir as mybir
from concourse import bacc
from concourse.bass_utils import run_bass_kernel_spmd
from concourse.masks import make_identity

BF16 = ml_dtypes.bfloat16

CORES = 8
B, S, D = 8, 32, 4096
H, KVH, HD = 32, 8, 128
SW = 4096
PREV = SW - S  # 4064
TOK = B * S  # 256
NH = H // KVH  # 4 Q heads per core
NHP = NH // 2  # head pairs per core
QCOLS = NH * HD  # 512 Q-projection columns per core
SCALE = float(HD) ** -0.5
NEG = -1.0e30

# hd permutation: interleaved (r0,i0,r1,i1,...) -> (r..., i...)
_IDX = np.concatenate([np.arange(0, HD, 2), np.arange(1, HD, 2)])

# exec time of the last traced run (ns), set when KERNEL_TRACE=1
LAST_EXEC_NS = None

_BUILD_CACHE = {}


def _install_ntff_hook():
    """Register the axon NTFF profiling hook (the agent image's antenv stub
    lacks axon_hooks). Only needed when tracing."""
    import types

    if "antenv.axon_hooks" in sys.modules:
        return
    try:
        from trn_agent_boot.trn_boot import _ntff_profile_via_ctypes

        hook = _ntff_profile_via_ctypes("/opt/axon/libaxon_pjrt.so")
    except Exception:
        hook = None
    mod = types.ModuleType("antenv.axon_hooks")
    mod._hook = hook
    mod.get_axon_ntff_profile_hook = lambda: mod._hook
    mod.set_axon_ntff_profile_hook = lambda h: setattr(mod, "_hook", h)
    sys.modules["antenv.axon_hooks"] = mod
    import antenv

    antenv.axon_hooks = mod


def build(d=D, prev=PREV, cores=CORES):
    """Build the per-core Bass graph."""
    assert d % 128 == 0 and d % cores == 0
    n_dc = d // 128  # contraction chunks for QKV projections
    n_hc = (H * HD) // 128  # contraction chunks for Wo (fixed head structure)
    outc = d // cores  # output columns per core (Wo column slice)
    n_cc = (prev + 127) // 128  # cached-KV chunks (last short)
    tail = prev - (n_cc - 1) * 128  # rows in last cached chunk (96)
    assert 0 < tail <= 128
    NEWC = TOK // 128  # new-token chunks (2)
    NT = n_cc + NEWC  # total softmax chunks (34)
    LAG = 2  # software-pipeline depth of the softmax chunk loop

    dt = mybir.dt
    bf, f32 = dt.bfloat16, dt.float32
    EXP = mybir.ActivationFunctionType.Exp
    ADD = mybir.AluOpType.add

    nc = bacc.Bacc("TRN2", target_bir_lowering=False, debug=False, num_devices=cores)

    xt_d = nc.dram_tensor("xt", [128, n_dc * TOK], bf, kind="ExternalInput")
    wq_d = nc.dram_tensor("wq", [128, NH * n_dc * HD], bf, kind="ExternalInput")
    wkv_d = nc.dram_tensor("wkv", [128, n_dc * 2 * HD], bf, kind="ExternalInput")
    kct_d = nc.dram_tensor("kct", [HD, prev], bf, kind="ExternalInput")
    vc_d = nc.dram_tensor("vc", [128, n_cc * HD], bf, kind="ExternalInput")
    wo_d = nc.dram_tensor("wo", [128, n_hc * outc], bf, kind="ExternalInput")
    cost_d = nc.dram_tensor("cost", [HD // 2, TOK], f32, kind="ExternalInput")
    sint_d = nc.dram_tensor("sint", [HD // 2, TOK], f32, kind="ExternalInput")
    maskm_d = nc.dram_tensor("maskm", [128, NEWC * TOK], f32, kind="ExternalInput")
    out_d = nc.dram_tensor("out", [TOK, outc], f32, kind="ExternalOutput")

    with tile.TileContext(nc) as tc, ExitStack() as ctx:
        from concourse.tile import add_dep_helper

        const = ctx.enter_context(tc.tile_pool(name="const", bufs=1))

        N_XP = 4  # xt DMA pieces (8 chunks each, 4KB partition lines)
        xt_sb = [const.tile([128, n_dc // N_XP, TOK], bf, tag=f"xt{i}", name=f"xt{i}")
                 for i in range(N_XP)]
        # per-head Wq, each in 2 pieces of 16 contraction chunks
        wqh_sb = [
            [const.tile([128, n_dc // 2, HD], bf, tag=f"wq{h}_{i}", name=f"wq{h}_{i}")
             for i in range(2)]
            for h in range(NH)
        ]
        wkv_sb = [const.tile([128, n_dc // 2, 2 * HD], bf, tag=f"wkv{i}", name=f"wkv{i}")
                  for i in range(2)]
        kct_sb = const.tile([128, prev], bf)
        vc_sb = const.tile([128, n_cc, HD], bf)
        wo_sb = const.tile([128, n_hc, outc], bf)
        cost_sb = const.tile([HD // 2, TOK], f32)
        sint_sb = const.tile([HD // 2, TOK], f32)
        maskm_sb = const.tile([128, NEWC, TOK], f32)
        ones_sb = const.tile([128, 1], bf)
        ident_sb = const.tile([128, 128], bf)
        qT_sb = [
            const.tile([128, 2, TOK], bf, tag=f"qT{p}", name=f"qT{p}")
            for p in range(NHP)
        ]
        kTn_sb = const.tile([128, TOK], bf)
        vnT_sb = const.tile([128, TOK], bf)
        vn_sb = const.tile([128, NEWC, HD], bf)  # new-token V, token-major
        recip_sb = [const.tile([1, 2 * TOK], f32, tag=f"rc{p}", name=f"rc{p}") for p in range(NHP)]
        recip_bc = [const.tile([128, 2 * TOK], f32, tag=f"rb{p}", name=f"rb{p}") for p in range(NHP)]
        attnout = [const.tile([128, 2 * TOK], bf, tag=f"ao{p}", name=f"ao{p}") for p in range(NHP)]
        all_sb = [
            [const.tile([128, 2 * TOK], bf, tag=f"all{p}_{r}", name=f"all{p}_{r}") for r in range(cores)]
            for p in range(NHP)
        ]
        out_sb = const.tile([128, 2, outc], f32, name="out_sb")

        # ---- constants built on-device ----
        nc.gpsimd.memset(ones_sb[:], 1.0)
        make_identity(nc, ident_sb[:])

        # ---- input DMAs. sync queue paces the projections (xt/wq pieces in
        # first-use order); scalar queue carries the attention-side tensors
        # (kct/vc/mask) in parallel. All host arrays are pre-permuted so each
        # SBUF partition line is one contiguous DRAM read.
        nc.scalar.dma_start(out=cost_sb[:], in_=cost_d.ap())
        nc.scalar.dma_start(out=sint_sb[:], in_=sint_d.ap())
        nc.scalar.dma_start(out=kct_sb[:], in_=kct_d.ap())
        nc.scalar.dma_start(
            out=vc_sb[:], in_=vc_d.ap().rearrange("p (c n) -> p c n", c=n_cc)
        )
        nc.scalar.dma_start(
            out=maskm_sb[:], in_=maskm_d.ap().rearrange("p (c n) -> p c n", c=NEWC)
        )
        xt_r = xt_d.ap().rearrange("p (c n) -> p c n", c=n_dc)
        wq_r = wq_d.ap().rearrange("p (h c j) -> p h c j", h=NH, c=n_dc)
        wkv_r = wkv_d.ap().rearrange("p (c n) -> p c n", c=n_dc)
        xp = n_dc // N_XP

        def dma_xt(i):
            nc.sync.dma_start(out=xt_sb[i][:], in_=xt_r[:, i * xp : (i + 1) * xp, :])

        def dma_wqh(h, i):
            nc.sync.dma_start(
                out=wqh_sb[h][i][:], in_=wq_r[:, h, i * 16 : (i + 1) * 16, :]
            )

        def dma_wkv(i):
            nc.sync.dma_start(
                out=wkv_sb[i][:], in_=wkv_r[:, i * 16 : (i + 1) * 16, :]
            )

        # interleave so q0 can start early and stay fed
        dma_xt(0); dma_wqh(0, 0); dma_xt(1); dma_wqh(0, 1)
        dma_xt(2); dma_wqh(1, 0); dma_xt(3); dma_wqh(1, 1)
        dma_wkv(0); dma_wkv(1); dma_wqh(2, 0); dma_wqh(2, 1); dma_wqh(3, 0); dma_wqh(3, 1)
        wo_r = wo_d.ap().rearrange("p (c n) -> p c n", c=n_hc)
        for i in range(4):
            sl = slice(i * 8, (i + 1) * 8)
            nc.sync.dma_start(out=wo_sb[:, sl, :], in_=wo_r[:, sl, :])

        rtmp = ctx.enter_context(tc.tile_pool(name="rope_tmp", bufs=4))

        def rope(src_ps, dst):
            hh = HD // 2
            qr, qi = src_ps[0:hh, :], src_ps[hh:128, :]
            t1 = rtmp.tile([hh, TOK], f32, tag="t1", name="t1")
            t2 = rtmp.tile([hh, TOK], f32, tag="t2", name="t2")
            nc.vector.tensor_mul(t1[:], qr, cost_sb[:])
            nc.vector.tensor_mul(t2[:], qi, sint_sb[:])
            nc.vector.tensor_sub(dst[0:hh, :], t1[:], t2[:])
            t3 = rtmp.tile([hh, TOK], f32, tag="t1", name="t1")
            t4 = rtmp.tile([hh, TOK], f32, tag="t2", name="t2")
            nc.vector.tensor_mul(t3[:], qr, sint_sb[:])
            nc.vector.tensor_mul(t4[:], qi, cost_sb[:])
            nc.vector.tensor_add(dst[hh:128, :], t3[:], t4[:])

        # ---- PE warmup: back-to-back matmuls so the HAM clock gate starts
        # ramping before the real work arrives ----
        warm_rhs = const.tile([128, 512], bf, name="warm_rhs")
        nc.vector.memset(warm_rhs[:], 0.0)
        with tc.tile_pool(name="warm_ps", bufs=1, space="PSUM") as warm_pool:
            wps = warm_pool.tile([128, 512], f32, tag="wps", name="wps")
            for _ in range(12):
                nc.tensor.matmul(
                    wps[:], warm_rhs[:, 0:128], warm_rhs[:],
                    start=True, stop=True, skip_group_check=True,
                )

        # ---- phase 1: q0/q1 projection up front (they gate pass 0);
        # V/K/q2/q3 projections are emitted later as filler work interleaved
        # into the pass-0 chunk loop so the PE never idles. ----
        def proj_mm(dst, lhs, c, st, sp):
            nc.tensor.matmul(
                dst, lhs, xt_sb[c // xp][:, c % xp, :],
                start=st, stop=sp, skip_group_check=True,
            )

        def proj_q_mm(h, dst, c):
            proj_mm(
                dst[:, 0:TOK],
                wqh_sb[h][c // 16][:, c % 16, :],
                c, c == 0, c == n_dc - 1,
            )

        with tc.tile_pool(name="proj_psA", bufs=1, space="PSUM") as proj_a:
            qkv_tiles = {
                i: proj_a.tile([128, 512], f32, tag=f"qkv{i}", name=f"qkv{i}")
                for i in (0, 1)
            }
            for h in (0, 1):
                for c in range(n_dc):
                    proj_q_mm(h, qkv_tiles[h], c)
                rope(qkv_tiles[h][:, 0:TOK], qT_sb[0][:, h, :])

        # ---- DRAM staging + collectives warmup (issued early so the
        # firmware is warm well before the first real AllGather) ----
        dram = ctx.enter_context(tc.tile_pool(name="dram", bufs=1, space="DRAM"))
        bar_src = const.tile([1, 32], bf, name="bar_src")
        nc.vector.memset(bar_src[:], 0.0)
        ag_in = [dram.tile([128, 2 * TOK], bf, tag=f"agi{p}", name=f"agi{p}") for p in range(NHP)]
        ag_out = [
            dram.tile(
                [128 * cores, 2 * TOK], bf, tag=f"ago{p}", name=f"ago{p}",
                addr_space="Shared",
            )
            for p in range(NHP)
        ]
        agw_in = dram.tile([1, 32], bf, name="agw_in")
        agw_out = dram.tile([cores, 32], bf, name="agw_out", addr_space="Shared")
        nc.gpsimd.dma_start(out=agw_in[:], in_=bar_src[:])
        nc.gpsimd.collective_compute(
            "AllGather",
            mybir.AluOpType.bypass,
            replica_groups=[list(range(cores))],
            ins=[agw_in.opt()],
            outs=[agw_out.opt()],
        )

        # softmax pools first, then filler pools on top of the pool stack
        # (released LIFO after pass 0) so PSUM stays within 8 banks:
        # 3 scores + 2 accum + 3 filler.
        s_pool = ctx.enter_context(tc.tile_pool(name="s_ps", bufs=3, space="PSUM"))
        acc_pool = ctx.enter_context(tc.tile_pool(name="acc_ps", bufs=1, space="PSUM"))
        attn_pool = ctx.enter_context(tc.tile_pool(name="attn", bufs=LAG + 4))
        fill_ctx = ExitStack()
        vk_pool = fill_ctx.enter_context(tc.tile_pool(name="vk_ps", bufs=1, space="PSUM"))
        q23_pool = fill_ctx.enter_context(tc.tile_pool(name="q23_ps", bufs=1, space="PSUM"))
        vt_pool = fill_ctx.enter_context(tc.tile_pool(name="vt_ps", bufs=1, space="PSUM"))

        # ---- filler closures: V proj -> V-new transposes -> K proj -> K rope
        # -> q2 proj+rope -> q3 proj+rope, consumed inside the pass-0 loop ----
        fill = []
        fstate = {}

        def F(fn):
            fill.append(fn)

        def vk_slice_mm(off, c):
            if c == 0:
                fstate["vk"] = vk_pool.tile([128, 512], f32, tag="vk", name="vk")
            proj_mm(
                fstate["vk"][:, 0:TOK],
                wkv_sb[c // 16][:, c % 16, off : off + HD],
                c, c == 0, c == n_dc - 1,
            )

        def vt_step(b):
            vt = vt_pool.tile([S, HD], bf, tag="vt", name="vt")
            nc.tensor.transpose(vt[:], vnT_sb[:, b * S : (b + 1) * S], ident_sb[:])
            nc.scalar.copy(vn_sb[(b % 4) * S : (b % 4) * S + S, b // 4, :], vt[:])

        def q23_mm(h, c):
            if c == 0:
                fstate[f"q{h}"] = q23_pool.tile([128, 512], f32, tag="q23", name="q23")
            proj_q_mm(h, fstate[f"q{h}"], c)

        for c in range(n_dc):
            F(lambda c=c: vk_slice_mm(HD, c))  # V proj
        F(lambda: nc.scalar.copy(vnT_sb[:], fstate["vk"][:, 0:TOK]))
        for b in range(B):
            F(lambda b=b: vt_step(b))
        for c in range(n_dc):
            F(lambda c=c: vk_slice_mm(0, c))  # K proj (reuses the V bank)
        F(lambda: rope(fstate["vk"][:, 0:TOK], kTn_sb))
        for h in (2, 3):
            for c in range(n_dc):
                F(lambda h=h, c=c: q23_mm(h, c))
            F(lambda h=h: rope(fstate[f"q{h}"][:, 0:TOK], qT_sb[1][:, h - 2, :]))

        # ---- phase 2+3: attention in two head-pair passes, AllGather each ----
        norm_gate = []  # pass-1 first scores matmul, for pass serialization
        last_av = None
        p0_norm = None

        def run_pass(p, filler, lag=LAG):
            nonlocal last_av, p0_norm
            qpair = qT_sb[p][:, :, :]  # [128, 2, TOK]
            o_ps = acc_pool.tile([128, 2, TOK], f32, tag="o", name="o")
            sum_ps = acc_pool.tile([1, 2, TOK], f32, tag="sum", name="sum")

            def flush(a_sb, n, t):
                nonlocal last_av
                nc.tensor.matmul(
                    sum_ps[0:1, :, :], ones_sb[0:n, 0:1], a_sb[0:n, :, :],
                    start=(t == 0), stop=(t == NT - 1), skip_group_check=True,
                )
                vw = vc_sb[0:n, t, :] if t < n_cc else vn_sb[:, t - n_cc, :]
                last_av = nc.tensor.matmul(
                    o_ps[:, :, :], vw, a_sb[0:n, :, :],
                    start=(t == 0), stop=(t == NT - 1), skip_group_check=True,
                )

            # uniform 34-chunk softmax loop, lag-LAG software pipeline.
            # filler work (V/K/q2/q3 projections) is drained by chunk 30 so
            # the new-token chunks (which need kTn/vn) see it completed.
            ndone = [0]
            work = []
            for t in range(NT):
                if t < n_cc:
                    n = 128 if t < n_cc - 1 else tail
                    wsl = kct_sb[:, t * 128 : t * 128 + n]
                else:
                    n = 128
                    wsl = kTn_sb[:, (t - n_cc) * 128 : (t - n_cc + 1) * 128]
                s_ps = s_pool.tile([128, 2, TOK], f32, tag="s", name="s")
                mm = nc.tensor.matmul(
                    s_ps[0:n, :, :], wsl, qpair,
                    start=True, stop=True, skip_group_check=True,
                )
                if p == 1 and t == 0:
                    norm_gate.append(mm)
                if t >= n_cc:
                    nc.vector.scalar_tensor_tensor(
                        out=s_ps[:, :, :],
                        in0=s_ps[:, :, :],
                        scalar=0.0,
                        in1=maskm_sb[:, t - n_cc, :]
                        .unsqueeze(1)
                        .broadcast_to((128, 2, TOK)),
                        op0=ADD,
                        op1=ADD,
                    )
                a_sb = attn_pool.tile([128, 2, TOK], bf, tag="a", name="a")
                nc.scalar.activation(a_sb[0:n, :, :], s_ps[0:n, :, :], EXP)
                work.append((a_sb, n, t))
                if len(work) > lag:
                    flush(*work.pop(0))
                if filler:
                    want = (len(filler) * (t + 1) + 29) // 30
                    while ndone[0] < min(want, len(filler)):
                        filler[ndone[0]]()
                        ndone[0] += 1
            for w in work:
                flush(*w)
            assert not filler or ndone[0] == len(filler)

            # 1/rowsum -> broadcast -> normalize on PSUM->SBUF copy
            nc.vector.reciprocal_approx_fast(
                recip_sb[p][:], sum_ps[0:1, :, :].rearrange("p h s -> p (h s)")
            )
            nc.gpsimd.partition_broadcast(recip_bc[p][:], recip_sb[p][:])
            norm = nc.vector.tensor_mul(
                attnout[p][:],
                o_ps[:, :, :].rearrange("p h s -> p (h s)"),
                recip_bc[p][:],
            )
            if p == 0:
                p0_norm = norm

            # AllGather this pass's heads (overlaps next pass's compute)
            nc.scalar.dma_start(ag_in[p][:], attnout[p][:])
            nc.gpsimd.collective_compute(
                "AllGather",
                mybir.AluOpType.bypass,
                replica_groups=[list(range(cores))],
                ins=[ag_in[p].opt()],
                outs=[ag_out[p].opt()],
            )
            ag_r = ag_out[p].rearrange("(r p) n -> p r n", p=128)
            for r in range(cores):
                nc.sync.dma_start(all_sb[p][r][:], ag_r[:, r, :])

        run_pass(0, fill)
        fill_ctx.close()
        run_pass(1, None, lag=3)

        # keep pass-1 scores behind pass-0's normalize so pass-0's AllGather
        # launches at the midpoint and overlaps pass-1 compute
        for mm in norm_gate:
            add_dep_helper(mm.ins, p0_norm.ins, sync=True, reason="serialize passes")

        # ---- phase 4: out = attnout_all @ Wo[:, slice], per pass ----
        wo_pool = ctx.enter_context(tc.tile_pool(name="wo_ps", bufs=1, space="PSUM"))
        out_ps = [wo_pool.tile([128, outc], f32, tag=f"out{k}", name=f"out{k}") for k in range(2)]
        for p in range(NHP):
            h0 = 2 * p
            for k in range(2):
                for r in range(cores):
                    for l in range(2):
                        g = r * NH + h0 + l
                        mm = nc.tensor.matmul(
                            out_ps[k][:],
                            all_sb[p][r][:, l * TOK + k * 128 : l * TOK + k * 128 + 128],
                            wo_sb[:, g, :],
                            start=(p == 0 and r == 0 and l == 0),
                            stop=(p == NHP - 1 and r == cores - 1 and l == 1),
                            skip_group_check=True,
                        )
                        if p == 0 and r == 0 and l == 0 and k == 0:
                            # keep Wo behind pass-1's attention in the PE
                            # stream (the scheduler's cost model underestimates
                            # the AllGather and would otherwise stall pass-1)
                            add_dep_helper(
                                mm.ins, last_av.ins, sync=True,
                                reason="Wo after pass-1 attention",
                            )

        # ---- output: PSUM -> SBUF -> DRAM ----
        out_r = out_d.ap().rearrange("(k p) n -> p k n", p=128)
        for k in range(2):
            nc.scalar.copy(out_sb[:, k, :], out_ps[k][:])
            nc.sync.dma_start(out_r[:, k, :], out_sb[:, k, :])

        if os.environ.get("KERNEL_DUMP_SLOTS", "0") == "1":
            dbg_d = nc.dram_tensor(
                "dbg", [128, NHP * cores * 2 * TOK], bf, kind="ExternalOutput"
            )
            dbg_r = dbg_d.ap().rearrange("p (q n) -> p q n", q=NHP * cores)
            for p in range(NHP):
                for r in range(cores):
                    nc.sync.dma_start(dbg_r[:, p * cores + r, :], all_sb[p][r][:])

    nc.compile()
    return nc


def _pmajor(a, nchunk):
    """[nchunk*128, F] -> [128, nchunk, F] (partition-major, C-contiguous)."""
    return np.ascontiguousarray(
        a.reshape(nchunk, 128, a.shape[-1]).transpose(1, 0, 2)
    )


def prep_in_maps(x, freqs_cos, freqs_sin, mask, cache_k, cache_v, Wq, Wk, Wv, Wo,
                 d=D, prev=PREV, cores=CORES):
    """Host-side sharding/layout. Returns in_maps for run_bass_kernel_spmd."""
    n_dc = d // 128
    n_cc = (prev + 127) // 128
    x = np.asarray(x, np.float32).reshape(TOK, d)
    xtp = _pmajor(np.ascontiguousarray(x.T), n_dc).astype(BF16)  # [128, 32, TOK]
    cost = np.ascontiguousarray(
        np.tile(np.asarray(freqs_cos, np.float32)[0].T, (1, B))
    )  # [64, TOK]
    sint = np.ascontiguousarray(
        np.tile(np.asarray(freqs_sin, np.float32)[0].T, (1, B))
    )
    # block-diagonal new-token mask: chunk j covers key tokens j*128..j*128+127
    mask = np.asarray(mask, np.float32)  # [B, Sq, Sk]
    NEWC = TOK // 128
    mm = np.full((NEWC, 128, TOK), NEG, np.float32)
    for j in range(NEWC):
        for pp in range(128):
            t = j * 128 + pp
            bk, sk = t // S, t % S
            mm[j, pp, bk * S : (bk + 1) * S] = mask[bk, :, sk]
    maskm = np.ascontiguousarray(mm.transpose(1, 0, 2))  # [128, NEWC, TOK]

    Wq = np.asarray(Wq, np.float32)
    Wk = np.asarray(Wk, np.float32)
    Wv = np.asarray(Wv, np.float32)
    Wo = np.asarray(Wo, np.float32)
    cache_k = np.asarray(cache_k, np.float32)
    cache_v = np.asarray(cache_v, np.float32)

    outc = d // cores
    in_maps = []
    for c in range(cores):
        wq_c = (Wq[:, c * QCOLS : (c + 1) * QCOLS] * SCALE).reshape(d, NH, HD)[
            :, :, _IDX
        ]  # [d, NH, HD]
        # [128, h, c, j] so each head's weights are one contiguous stream
        wqp = np.ascontiguousarray(
            wq_c.reshape(n_dc, 128, NH, HD).transpose(1, 2, 0, 3)
        )
        wk_c = Wk[:, c * HD : (c + 1) * HD][:, _IDX]
        wv_c = Wv[:, c * HD : (c + 1) * HD]
        wkv_c = np.concatenate([wk_c, wv_c], axis=1)  # [d, 256]
        wkvp = _pmajor(wkv_c, n_dc)
        kct_c = np.ascontiguousarray(cache_k[0, :prev, c, :][:, _IDX].T)  # [HD, prev]
        vc_pad = np.zeros((n_cc * 128, HD), np.float32)
        vc_pad[0:prev] = cache_v[0, :prev, c, :]
        vcp = _pmajor(vc_pad, n_cc)  # [128, 32, 128]
        wo_c = Wo[:, c * outc : (c + 1) * outc]
        wop = _pmajor(wo_c, (H * HD) // 128)  # [128, 32, outc]
        in_maps.append(
            {
                "xt": xtp.reshape(128, -1),
                "wq": wqp.astype(BF16).reshape(128, -1),
                "wkv": wkvp.astype(BF16).reshape(128, -1),
                "kct": kct_c.astype(BF16),
                "vc": vcp.astype(BF16).reshape(128, -1),
                "wo": wop.astype(BF16).reshape(128, -1),
                "cost": cost,
                "sint": sint,
                "maskm": maskm.reshape(128, -1),
            }
        )
    return in_maps


def kernel(x, freqs_cos, freqs_sin, mask, cache_k, cache_v, Wq, Wk, Wv, Wo, positions):
    global LAST_EXEC_NS
    assert int(positions) == PREV, f"kernel compiled for positions={PREV}"

    key = (D, PREV)
    if key not in _BUILD_CACHE:
        _BUILD_CACHE[key] = build(D, PREV, CORES)
    nc = _BUILD_CACHE[key]

    in_maps = prep_in_maps(
        x, freqs_cos, freqs_sin, mask, cache_k, cache_v, Wq, Wk, Wv, Wo
    )

    trace = os.environ.get("KERNEL_TRACE", "0") == "1"
    if trace:
        _install_ntff_hook()
    res = run_bass_kernel_spmd(
        nc, in_maps, core_ids=list(range(CORES)), trace=trace
    )
    if trace:
        LAST_EXEC_NS = res.exec_time_ns

    outc = D // CORES
    out = np.empty((TOK, D), np.float32)
    for c in range(CORES):
        out[:, c * outc : (c + 1) * outc] = res.results[c]["out"]
    return out.reshape(B, S, D)


# revision 45
# speedup vs baseline: 1.1589x; 1.1589x over previous
"""Trainium2 Bass kernel for GQA sliding-window attention (8-core SPMD).

Problem: B=8, S=32, D=4096, H=32 Q-heads, KVH=8 KV-heads, HD=128,
sliding window 4096 with 4064 cached positions.

Sharding: tensor-parallel over heads. Core c owns Q heads 4c..4c+3 and KV
head c (one GQA group): Wq/Wk/Wv column-sharded, cache sharded by KV head,
x replicated. Each core computes its 4 heads' attention output in two
head-pair passes; after each pass the cores AllGather that pass's (bf16)
attention outputs so the gather overlaps the next pass's compute. Each core
then applies a column slice of Wo, and the host concatenates column slices
(no host-side arithmetic).

Compute is bf16 on the TensorEngine (fp32 PSUM accumulation, fp32 softmax
intermediates). Host-side sharding also does layout prep:
  - x is fed transposed (xT [D, 256]) so QKV projections produce Q^T/K^T
    directly in [head_dim, token] layout.
  - All large inputs are pre-permuted on the host so every DMA writes each
    SBUF partition from one contiguous DRAM line (2-16KB), not 512B strides.
  - Wq/Wk columns (and cached K's hd axis) are permuted so RoPE's interleaved
    (even,odd) pairs become contiguous halves [0:64]=real, [64:128]=imag.
    The permutation cancels in q.k since both sides share it; V/Wo stay
    unpermuted.
  - SCALE = HD^-0.5 is folded into Wq.

The new-token scores are folded into the cached-KV chunk loop as two extra
128-token chunks with a host-built block-diagonal mask (-1e30 off-diagonal,
user mask on the diagonal), so the whole softmax runs as one uniform
34-chunk software pipeline (lag-2: exp of chunk t overlaps scores of t+1
and sum/AV of t-2).

Softmax skips max-subtraction (scores are O(10), exp is safe in fp32) and
normalization is deferred: unnormalized exp(scores) feeds attn@V, row sums
come from a ones-vector matmul, and 1/sum is applied when copying the
attention output out of PSUM.
"""

import os
import sys
from contextlib import ExitStack

import numpy as np
import ml_dtypes

import concourse.bass as bass
import concourse.tile as tile
import concourse.mybir as mybir
from concourse import bacc
from concourse.bass_utils import run_bass_kernel_spmd
from concourse.masks import make_identity

BF16 = ml_dtypes.bfloat16

CORES = 8
B, S, D = 8, 32, 4096
H, KVH, HD = 32, 8, 128
SW = 4096
PREV = SW - S  # 4064
TOK = B * S  # 256
NH = H // KVH  # 4 Q heads per core
NHP = NH // 2  # head pairs per core
QCOLS = NH * HD  # 512 Q-projection columns per core
SCALE = float(HD) ** -0.5
NEG = -1.0e30

# hd permutation: interleaved (r0,i0,r1,i1,...) -> (r..., i...)
_IDX = np.concatenate([np.arange(0, HD, 2), np.arange(1, HD, 2)])

# exec time of the last traced run (ns), set when KERNEL_TRACE=1
LAST_EXEC_NS = None

_BUILD_CACHE = {}


def _install_ntff_hook():
    """Register the axon NTFF profiling hook (the agent image's antenv stub
    lacks axon_hooks). Only needed when tracing."""
    import types

    if "antenv.axon_hooks" in sys.modules:
        return
    try:
        from trn_agent_boot.trn_boot import _ntff_profile_via_ctypes

        hook = _ntff_profile_via_ctypes("/opt/axon/libaxon_pjrt.so")
    except Exception:
        hook = None
    mod = types.ModuleType("antenv.axon_hooks")
    mod._hook = hook
    mod.get_axon_ntff_profile_hook = lambda: mod._hook
    mod.set_axon_ntff_profile_hook = lambda h: setattr(mod, "_hook", h)
    sys.modules["antenv.axon_hooks"] = mod
    import antenv

    antenv.axon_hooks = mod


def build(d=D, prev=PREV, cores=CORES):
    """Build the per-core Bass graph."""
    assert d % 128 == 0 and d % cores == 0
    n_dc = d // 128  # contraction chunks for QKV projections
    n_hc = (H * HD) // 128  # contraction chunks for Wo (fixed head structure)
    outc = d // cores  # output columns per core (Wo column slice)
    n_cc = (prev + 127) // 128  # cached-KV chunks (last short)
    tail = prev - (n_cc - 1) * 128  # rows in last cached chunk (96)
    assert 0 < tail <= 128
    NEWC = TOK // 128  # new-token chunks (2)
    NT = n_cc + NEWC  # total softmax chunks (34)
    LAG = 2  # software-pipeline depth of the softmax chunk loop

    dt = mybir.dt
    bf, f32 = dt.bfloat16, dt.float32
    EXP = mybir.ActivationFunctionType.Exp
    ADD = mybir.AluOpType.add

    nc = bacc.Bacc("TRN2", target_bir_lowering=False, debug=False, num_devices=cores)

    xt_d = nc.dram_tensor("xt", [128, n_dc * TOK], bf, kind="ExternalInput")
    wq_d = nc.dram_tensor("wq", [128, NH * n_dc * HD], bf, kind="ExternalInput")
    wkv_d = nc.dram_tensor("wkv", [128, n_dc * 2 * HD], bf, kind="ExternalInput")
    kct_d = nc.dram_tensor("kct", [HD, prev], bf, kind="ExternalInput")
    vc_d = nc.dram_tensor("vc", [128, n_cc * HD], bf, kind="ExternalInput")
    wo_d = nc.dram_tensor("wo", [128, n_hc * outc], bf, kind="ExternalInput")
    cost_d = nc.dram_tensor("cost", [HD // 2, TOK], f32, kind="ExternalInput")
    sint_d = nc.dram_tensor("sint", [HD // 2, TOK], f32, kind="ExternalInput")
    maskm_d = nc.dram_tensor("maskm", [128, NEWC * TOK], f32, kind="ExternalInput")
    out_d = nc.dram_tensor("out", [TOK, outc], f32, kind="ExternalOutput")

    with tile.TileContext(nc) as tc, ExitStack() as ctx:
        from concourse.tile import add_dep_helper

        const = ctx.enter_context(tc.tile_pool(name="const", bufs=1))

        N_XP = 4  # xt DMA pieces (8 chunks each, 4KB partition lines)
        xt_sb = [const.tile([128, n_dc // N_XP, TOK], bf, tag=f"xt{i}", name=f"xt{i}")
                 for i in range(N_XP)]
        # per-head Wq, each in 2 pieces of 16 contraction chunks
        wqh_sb = [
            [const.tile([128, n_dc // 2, HD], bf, tag=f"wq{h}_{i}", name=f"wq{h}_{i}")
             for i in range(2)]
            for h in range(NH)
        ]
        wkv_sb = [const.tile([128, n_dc // 2, 2 * HD], bf, tag=f"wkv{i}", name=f"wkv{i}")
                  for i in range(2)]
        kct_sb = const.tile([128, prev], bf)
        vc_sb = const.tile([128, n_cc, HD], bf)
        wo_sb = const.tile([128, n_hc, outc], bf)
        cost_sb = const.tile([HD // 2, TOK], f32)
        sint_sb = const.tile([HD // 2, TOK], f32)
        maskm_sb = const.tile([128, NEWC, TOK], f32)
        ones_sb = const.tile([128, 1], bf)
        ones32_sb = const.tile([128, 1], f32)
        ident_sb = const.tile([128, 128], bf)
        qT_sb = [
            const.tile([128, 2, TOK], bf, tag=f"qT{p}", name=f"qT{p}")
            for p in range(NHP)
        ]
        kTn_sb = const.tile([128, TOK], bf)
        vnT_sb = const.tile([128, TOK], bf)
        vn_sb = const.tile([128, NEWC, HD], bf)  # new-token V, token-major
        recip_sb = [const.tile([1, 2 * TOK], f32, tag=f"rc{p}", name=f"rc{p}") for p in range(NHP)]
        recip_bc = [const.tile([128, 2 * TOK], f32, tag=f"rb{p}", name=f"rb{p}") for p in range(NHP)]
        attnout = [const.tile([128, 2 * TOK], bf, tag=f"ao{p}", name=f"ao{p}") for p in range(NHP)]
        accv_sb = [const.tile([128, 2, TOK], f32, tag=f"acv{p}", name=f"acv{p}") for p in range(NHP)]
        all_sb = [
            [const.tile([128, 2 * TOK], bf, tag=f"all{p}_{r}", name=f"all{p}_{r}") for r in range(cores)]
            for p in range(NHP)
        ]
        out_sb = const.tile([128, 2, outc], f32, name="out_sb")

        # ---- constants built on-device ----
        nc.gpsimd.memset(ones_sb[:], 1.0)
        nc.gpsimd.memset(ones32_sb[:], 1.0)
        make_identity(nc, ident_sb[:])

        # ---- input DMAs. sync queue paces the projections (xt/wq pieces in
        # first-use order); scalar queue carries the attention-side tensors
        # (kct/vc/mask) in parallel. All host arrays are pre-permuted so each
        # SBUF partition line is one contiguous DRAM read.
        nc.scalar.dma_start(out=cost_sb[:], in_=cost_d.ap())
        nc.scalar.dma_start(out=sint_sb[:], in_=sint_d.ap())
        nc.scalar.dma_start(out=kct_sb[:], in_=kct_d.ap())
        nc.scalar.dma_start(
            out=vc_sb[:], in_=vc_d.ap().rearrange("p (c n) -> p c n", c=n_cc)
        )
        nc.scalar.dma_start(
            out=maskm_sb[:], in_=maskm_d.ap().rearrange("p (c n) -> p c n", c=NEWC)
        )
        xt_r = xt_d.ap().rearrange("p (c n) -> p c n", c=n_dc)
        wq_r = wq_d.ap().rearrange("p (h c j) -> p h c j", h=NH, c=n_dc)
        wkv_r = wkv_d.ap().rearrange("p (c n) -> p c n", c=n_dc)
        xp = n_dc // N_XP

        def dma_xt(i):
            nc.sync.dma_start(out=xt_sb[i][:], in_=xt_r[:, i * xp : (i + 1) * xp, :])

        def dma_wqh(h, i):
            nc.sync.dma_start(
                out=wqh_sb[h][i][:], in_=wq_r[:, h, i * 16 : (i + 1) * 16, :]
            )

        def dma_wkv(i):
            nc.sync.dma_start(
                out=wkv_sb[i][:], in_=wkv_r[:, i * 16 : (i + 1) * 16, :]
            )

        # interleave so q0 can start early and stay fed
        dma_xt(0); dma_wqh(0, 0); dma_xt(1); dma_wqh(0, 1)
        dma_xt(2); dma_wqh(1, 0); dma_xt(3); dma_wqh(1, 1)
        dma_wkv(0); dma_wkv(1); dma_wqh(2, 0); dma_wqh(2, 1); dma_wqh(3, 0); dma_wqh(3, 1)
        wo_r = wo_d.ap().rearrange("p (c n) -> p c n", c=n_hc)
        for i in range(4):
            sl = slice(i * 8, (i + 1) * 8)
            nc.sync.dma_start(out=wo_sb[:, sl, :], in_=wo_r[:, sl, :])

        rtmp = ctx.enter_context(tc.tile_pool(name="rope_tmp", bufs=4))

        def rope(src_ps, dst):
            hh = HD // 2
            qr, qi = src_ps[0:hh, :], src_ps[hh:128, :]
            t1 = rtmp.tile([hh, TOK], f32, tag="t1", name="t1")
            t2 = rtmp.tile([hh, TOK], f32, tag="t2", name="t2")
            nc.vector.tensor_mul(t1[:], qr, cost_sb[:])
            nc.vector.tensor_mul(t2[:], qi, sint_sb[:])
            nc.vector.tensor_sub(dst[0:hh, :], t1[:], t2[:])
            t3 = rtmp.tile([hh, TOK], f32, tag="t1", name="t1")
            t4 = rtmp.tile([hh, TOK], f32, tag="t2", name="t2")
            nc.vector.tensor_mul(t3[:], qr, sint_sb[:])
            nc.vector.tensor_mul(t4[:], qi, cost_sb[:])
            nc.vector.tensor_add(dst[hh:128, :], t3[:], t4[:])

        # ---- PE warmup: back-to-back matmuls so the HAM clock gate starts
        # ramping before the real work arrives ----
        warm_rhs = const.tile([128, 512], bf, name="warm_rhs")
        nc.vector.memset(warm_rhs[:], 0.0)
        with tc.tile_pool(name="warm_ps", bufs=1, space="PSUM") as warm_pool:
            wps = warm_pool.tile([128, 512], f32, tag="wps", name="wps")
            for _ in range(12):
                nc.tensor.matmul(
                    wps[:], warm_rhs[:, 0:128], warm_rhs[:],
                    start=True, stop=True, skip_group_check=True,
                )

        # ---- phase 1: q0/q1 projection up front (they gate pass 0);
        # V/K/q2/q3 projections are emitted later as filler work interleaved
        # into the pass-0 chunk loop so the PE never idles. ----
        def proj_mm(dst, lhs, c, st, sp):
            nc.tensor.matmul(
                dst, lhs, xt_sb[c // xp][:, c % xp, :],
                start=st, stop=sp, skip_group_check=True,
            )

        def proj_q_mm(h, dst, c):
            proj_mm(
                dst[:, 0:TOK],
                wqh_sb[h][c // 16][:, c % 16, :],
                c, c == 0, c == n_dc - 1,
            )

        with tc.tile_pool(name="proj_psA", bufs=1, space="PSUM") as proj_a:
            qkv_tiles = {
                i: proj_a.tile([128, 512], f32, tag=f"qkv{i}", name=f"qkv{i}")
                for i in (0, 1)
            }
            for h in (0, 1):
                for c in range(n_dc):
                    proj_q_mm(h, qkv_tiles[h], c)
                rope(qkv_tiles[h][:, 0:TOK], qT_sb[0][:, h, :])

        # ---- DRAM staging + collectives warmup (issued early so the
        # firmware is warm well before the first real AllGather) ----
        dram = ctx.enter_context(tc.tile_pool(name="dram", bufs=1, space="DRAM"))
        bar_src = const.tile([1, 32], bf, name="bar_src")
        nc.vector.memset(bar_src[:], 0.0)
        ag_in = [dram.tile([128, 2 * TOK], bf, tag=f"agi{p}", name=f"agi{p}") for p in range(NHP)]
        ag_out = [
            dram.tile(
                [128 * cores, 2 * TOK], bf, tag=f"ago{p}", name=f"ago{p}",
                addr_space="Shared",
            )
            for p in range(NHP)
        ]
        agw_in = dram.tile([1, 32], bf, name="agw_in")
        agw_out = dram.tile([cores, 32], bf, name="agw_out", addr_space="Shared")
        nc.gpsimd.dma_start(out=agw_in[:], in_=bar_src[:])
        nc.gpsimd.collective_compute(
            "AllGather",
            mybir.AluOpType.bypass,
            replica_groups=[list(range(cores))],
            ins=[agw_in.opt()],
            outs=[agw_out.opt()],
        )

        # softmax pools first, then filler pools on top of the pool stack
        # (released LIFO after pass 0) so PSUM stays within 8 banks:
        # 3 scores + 2 accum + 3 filler.
        s_pool = ctx.enter_context(tc.tile_pool(name="s_ps", bufs=3, space="PSUM"))
        acc_pool = ctx.enter_context(tc.tile_pool(name="acc_ps", bufs=1, space="PSUM"))
        attn_pool = ctx.enter_context(tc.tile_pool(name="attn", bufs=LAG + 4))
        fill_ctx = ExitStack()
        vk_pool = fill_ctx.enter_context(tc.tile_pool(name="vk_ps", bufs=1, space="PSUM"))
        q23_pool = fill_ctx.enter_context(tc.tile_pool(name="q23_ps", bufs=1, space="PSUM"))
        vt_pool = fill_ctx.enter_context(tc.tile_pool(name="vt_ps", bufs=1, space="PSUM"))

        # ---- filler closures: V proj -> V-new transposes -> K proj -> K rope
        # -> q2 proj+rope -> q3 proj+rope, consumed inside the pass-0 loop ----
        fill = []
        fstate = {}

        def F(fn):
            fill.append(fn)

        def vk_slice_mm(off, c):
            if c == 0:
                fstate["vk"] = vk_pool.tile([128, 512], f32, tag="vk", name="vk")
            proj_mm(
                fstate["vk"][:, 0:TOK],
                wkv_sb[c // 16][:, c % 16, off : off + HD],
                c, c == 0, c == n_dc - 1,
            )

        def vt_step(b):
            vt = vt_pool.tile([S, HD], bf, tag="vt", name="vt")
            nc.tensor.transpose(vt[:], vnT_sb[:, b * S : (b + 1) * S], ident_sb[:])
            nc.scalar.copy(vn_sb[(b % 4) * S : (b % 4) * S + S, b // 4, :], vt[:])

        def q23_mm(h, c):
            if c == 0:
                fstate[f"q{h}"] = q23_pool.tile([128, 512], f32, tag="q23", name="q23")
            proj_q_mm(h, fstate[f"q{h}"], c)

        for c in range(n_dc):
            F(lambda c=c: vk_slice_mm(HD, c))  # V proj
        F(lambda: nc.scalar.copy(vnT_sb[:], fstate["vk"][:, 0:TOK]))
        for b in range(B):
            F(lambda b=b: vt_step(b))
        for c in range(n_dc):
            F(lambda c=c: vk_slice_mm(0, c))  # K proj (reuses the V bank)
        F(lambda: rope(fstate["vk"][:, 0:TOK], kTn_sb))
        for h in (2, 3):
            for c in range(n_dc):
                F(lambda h=h, c=c: q23_mm(h, c))
            F(lambda h=h: rope(fstate[f"q{h}"][:, 0:TOK], qT_sb[1][:, h - 2, :]))

        # ---- phase 2+3: attention in two head-pair passes, AllGather each ----
        norm_gate = []  # pass-1 first scores matmul, for pass serialization
        last_av = None
        p0_norm = None

        def run_pass(p, filler, lag=LAG):
            nonlocal last_av, p0_norm
            qpair = qT_sb[p][:, :, :]  # [128, 2, TOK]
            o_ps = acc_pool.tile([128, 2, TOK], f32, tag="o", name="o")
            sum_ps = acc_pool.tile([1, 2, TOK], f32, tag="sum", name="sum")
            # rowsums: elementwise-accumulate exp chunks on the (otherwise
            # idle) vector engine, then one fp32 ones-matmul at pass end —
            # keeps the per-chunk PE work to scores+AV only
            acc_sb = accv_sb[p]
            nc.vector.memset(acc_sb[:], 0.0)

            def flush(a_sb, n, t):
                nonlocal last_av
                vw = vc_sb[0:n, t, :] if t < n_cc else vn_sb[:, t - n_cc, :]
                last_av = nc.tensor.matmul(
                    o_ps[:, :, :], vw, a_sb[0:n, :, :],
                    start=(t == 0), stop=(t == NT - 1), skip_group_check=True,
                )

            # uniform 34-chunk softmax loop, lag-LAG software pipeline.
            # filler work (V/K/q2/q3 projections) is drained by chunk 30 so
            # the new-token chunks (which need kTn/vn) see it completed.
            ndone = [0]
            work = []
            for t in range(NT):
                if t < n_cc:
                    n = 128 if t < n_cc - 1 else tail
                    wsl = kct_sb[:, t * 128 : t * 128 + n]
                else:
                    n = 128
                    wsl = kTn_sb[:, (t - n_cc) * 128 : (t - n_cc + 1) * 128]
                s_ps = s_pool.tile([128, 2, TOK], f32, tag="s", name="s")
                mm = nc.tensor.matmul(
                    s_ps[0:n, :, :], wsl, qpair,
                    start=True, stop=True, skip_group_check=True,
                )
                if p == 1 and t == 0:
                    norm_gate.append(mm)
                if t >= n_cc:
                    nc.vector.scalar_tensor_tensor(
                        out=s_ps[:, :, :],
                        in0=s_ps[:, :, :],
                        scalar=0.0,
                        in1=maskm_sb[:, t - n_cc, :]
                        .unsqueeze(1)
                        .broadcast_to((128, 2, TOK)),
                        op0=ADD,
                        op1=ADD,
                    )
                a_sb = attn_pool.tile([128, 2, TOK], bf, tag="a", name="a")
                nc.scalar.activation(a_sb[0:n, :, :], s_ps[0:n, :, :], EXP)
                nc.vector.tensor_add(
                    acc_sb[0:n, :, :], acc_sb[0:n, :, :], a_sb[0:n, :, :]
                )
                work.append((a_sb, n, t))
                if len(work) > lag:
                    flush(*work.pop(0))
                if filler:
                    want = (len(filler) * (t + 1) + 29) // 30
                    while ndone[0] < min(want, len(filler)):
                        filler[ndone[0]]()
                        ndone[0] += 1
            for w in work:
                flush(*w)
            assert not filler or ndone[0] == len(filler)

            # single fp32 reduction of the accumulated exp sums
            nc.tensor.matmul(
                sum_ps[0:1, :, :], ones32_sb[:, 0:1], acc_sb[:, :, :],
                start=True, stop=True, skip_group_check=True,
            )

            # 1/rowsum -> broadcast -> normalize on PSUM->SBUF copy
            nc.vector.reciprocal_approx_fast(
                recip_sb[p][:], sum_ps[0:1, :, :].rearrange("p h s -> p (h s)")
            )
            nc.gpsimd.partition_broadcast(recip_bc[p][:], recip_sb[p][:])
            norm = nc.vector.tensor_mul(
                attnout[p][:],
                o_ps[:, :, :].rearrange("p h s -> p (h s)"),
                recip_bc[p][:],
            )
            if p == 0:
                p0_norm = norm

            # AllGather this pass's heads (overlaps next pass's compute)
            nc.scalar.dma_start(ag_in[p][:], attnout[p][:])
            nc.gpsimd.collective_compute(
                "AllGather",
                mybir.AluOpType.bypass,
                replica_groups=[list(range(cores))],
                ins=[ag_in[p].opt()],
                outs=[ag_out[p].opt()],
            )
            ag_r = ag_out[p].rearrange("(r p) n -> p r n", p=128)
            for r in range(cores):
                nc.sync.dma_start(all_sb[p][r][:], ag_r[:, r, :])

        run_pass(0, fill)
        fill_ctx.close()
        run_pass(1, None, lag=3)

        # keep pass-1 scores behind pass-0's normalize so pass-0's AllGather
        # launches at the midpoint and overlaps pass-1 compute
        for mm in norm_gate:
            add_dep_helper(mm.ins, p0_norm.ins, sync=True, reason="serialize passes")

        # ---- phase 4: out = attnout_all @ Wo[:, slice], per pass ----
        wo_pool = ctx.enter_context(tc.tile_pool(name="wo_ps", bufs=1, space="PSUM"))
        out_ps = [wo_pool.tile([128, outc], f32, tag=f"out{k}", name=f"out{k}") for k in range(2)]
        for p in range(NHP):
            h0 = 2 * p
            for k in range(2):
                for r in range(cores):
                    for l in range(2):
                        g = r * NH + h0 + l
                        mm = nc.tensor.matmul(
                            out_ps[k][:],
                            all_sb[p][r][:, l * TOK + k * 128 : l * TOK + k * 128 + 128],
                            wo_sb[:, g, :],
                            start=(p == 0 and r == 0 and l == 0),
                            stop=(p == NHP - 1 and r == cores - 1 and l == 1),
                            skip_group_check=True,
                        )
                        if p == 0 and r == 0 and l == 0 and k == 0:
                            # keep Wo behind pass-1's attention in the PE
                            # stream (the scheduler's cost model underestimates
                            # the AllGather and would otherwise stall pass-1)
                            add_dep_helper(
                                mm.ins, last_av.ins, sync=True,
                                reason="Wo after pass-1 attention",
                            )

        # ---- output: PSUM -> SBUF -> DRAM ----
        out_r = out_d.ap().rearrange("(k p) n -> p k n", p=128)
        for k in range(2):
            nc.scalar.copy(out_sb[:, k, :], out_ps[k][:])
            nc.sync.dma_start(out_r[:, k, :], out_sb[:, k, :])

        if os.environ.get("KERNEL_DUMP_SLOTS", "0") == "1":
            dbg_d = nc.dram_tensor(
                "dbg", [128, NHP * cores * 2 * TOK], bf, kind="ExternalOutput"
            )
            dbg_r = dbg_d.ap().rearrange("p (q n) -> p q n", q=NHP * cores)
            for p in range(NHP):
                for r in range(cores):
                    nc.sync.dma_start(dbg_r[:, p * cores + r, :], all_sb[p][r][:])

    nc.compile()
    return nc


def _pmajor(a, nchunk):
    """[nchunk*128, F] -> [128, nchunk, F] (partition-major, C-contiguous)."""
    return np.ascontiguousarray(
        a.reshape(nchunk, 128, a.shape[-1]).transpose(1, 0, 2)
    )


def prep_in_maps(x, freqs_cos, freqs_sin, mask, cache_k, cache_v, Wq, Wk, Wv, Wo,
                 d=D, prev=PREV, cores=CORES):
    """Host-side sharding/layout. Returns in_maps for run_bass_kernel_spmd."""
    n_dc = d // 128
    n_cc = (prev + 127) // 128
    x = np.asarray(x, np.float32).reshape(TOK, d)
    xtp = _pmajor(np.ascontiguousarray(x.T), n_dc).astype(BF16)  # [128, 32, TOK]
    cost = np.ascontiguousarray(
        np.tile(np.asarray(freqs_cos, np.float32)[0].T, (1, B))
    )  # [64, TOK]
    sint = np.ascontiguousarray(
        np.tile(np.asarray(freqs_sin, np.float32)[0].T, (1, B))
    )
    # block-diagonal new-token mask: chunk j covers key tokens j*128..j*128+127
    mask = np.asarray(mask, np.float32)  # [B, Sq, Sk]
    NEWC = TOK // 128
    mm = np.full((NEWC, 128, TOK), NEG, np.float32)
    for j in range(NEWC):
        for pp in range(128):
            t = j * 128 + pp
            bk, sk = t // S, t % S
            mm[j, pp, bk * S : (bk + 1) * S] = mask[bk, :, sk]
    maskm = np.ascontiguousarray(mm.transpose(1, 0, 2))  # [128, NEWC, TOK]

    Wq = np.asarray(Wq, np.float32)
    Wk = np.asarray(Wk, np.float32)
    Wv = np.asarray(Wv, np.float32)
    Wo = np.asarray(Wo, np.float32)
    cache_k = np.asarray(cache_k, np.float32)
    cache_v = np.asarray(cache_v, np.float32)

    outc = d // cores
    in_maps = []
    for c in range(cores):
        wq_c = (Wq[:, c * QCOLS : (c + 1) * QCOLS] * SCALE).reshape(d, NH, HD)[
            :, :, _IDX
        ]  # [d, NH, HD]
        # [128, h, c, j] so each head's weights are one contiguous stream
        wqp = np.ascontiguousarray(
            wq_c.reshape(n_dc, 128, NH, HD).transpose(1, 2, 0, 3)
        )
        wk_c = Wk[:, c * HD : (c + 1) * HD][:, _IDX]
        wv_c = Wv[:, c * HD : (c + 1) * HD]
        wkv_c = np.concatenate([wk_c, wv_c], axis=1)  # [d, 256]
        wkvp = _pmajor(wkv_c, n_dc)
        kct_c = np.ascontiguousarray(cache_k[0, :prev, c, :][:, _IDX].T)  # [HD, prev]
        vc_pad = np.zeros((n_cc * 128, HD), np.float32)
        vc_pad[0:prev] = cache_v[0, :prev, c, :]
        vcp = _pmajor(vc_pad, n_cc)  # [128, 32, 128]
        wo_c = Wo[:, c * outc : (c + 1) * outc]
        wop = _pmajor(wo_c, (H * HD) // 128)  # [128, 32, outc]
        in_maps.append(
            {
                "xt": xtp.reshape(128, -1),
                "wq": wqp.astype(BF16).reshape(128, -1),
                "wkv": wkvp.astype(BF16).reshape(128, -1),
                "kct": kct_c.astype(BF16),
                "vc": vcp.astype(BF16).reshape(128, -1),
                "wo": wop.astype(BF16).reshape(128, -1),
                "cost": cost,
                "sint": sint,
                "maskm": maskm.reshape(128, -1),
            }
        )
    return in_maps


def kernel(x, freqs_cos, freqs_sin, mask, cache_k, cache_v, Wq, Wk, Wv, Wo, positions):
    global LAST_EXEC_NS
    assert int(positions) == PREV, f"kernel compiled for positions={PREV}"

    key = (D, PREV)
    if key not in _BUILD_CACHE:
        _BUILD_CACHE[key] = build(D, PREV, CORES)
    nc = _BUILD_CACHE[key]

    in_maps = prep_in_maps(
        x, freqs_cos, freqs_sin, mask, cache_k, cache_v, Wq, Wk, Wv, Wo
    )

    trace = os.environ.get("KERNEL_TRACE", "0") == "1"
    if trace:
        _install_ntff_hook()
    res = run_bass_kernel_spmd(
        nc, in_maps, core_ids=list(range(CORES)), trace=trace
    )
    if trace:
        LAST_EXEC_NS = res.exec_time_ns

    outc = D // CORES
    out = np.empty((TOK, D), np.float32)
    for c in range(CORES):
        out[:, c * outc : (c + 1) * outc] = res.results[c]["out"]
    return out.reshape(B, S, D)
